# revision 7
# baseline (speedup 1.0000x reference)
"""Multi-head causal attention (B=4, T=2048, C=1024, H=16, D=64) on 8 trn2 cores.

Sharding: core c owns batch b = c//2 and heads g*8..g*8+7 where g = c%2
(batch-parallel x head-tensor-parallel). Each core computes its 8 heads'
QKV projections, causal attention, and a partial output projection
(columns of Wp belonging to its heads). Host sums the two head-group
partials per batch and adds the bias.

Precision/throughput scheme (fp8 = e4m3, DoubleRow = 2 contraction rows
per partition at 0.5 PE cycles/out-col):
  QKV:    psum = x8.W8 + rx.W8h + x8.rWh   (3-pass residual-compensated
          fp8 DoubleRow over 256-deep super k-tiles; W8 = fp8(32W),
          rx = fp8(32(x-x8)), W8h = fp8(W8/32), rWh = fp8(32W - W8))
          -> psum ~= 32 * (x . W), near-bf16 accuracy at 6/8 of f32r cost.
  scores: q8 = fp8(qkv_psum) [~32q]; per-strip DoubleRow with d=64 folded
          to [32 partitions x 2 slots]; exp scale 1/8192 recovers
          softmax(q.k/sqrt(D)). Causal masking via additive -1e6 on the
          diagonal psum block BEFORE exp (no post-exp mask multiply).
  AV:     tier B: strips+V in bf16, [V|32] stationary (rowsum via the
          32-valued ones column; matches the 32x v scale).
          tier C (AV_FP8): strips fp8; stationary DoubleRow slots
          (v8, rv) with rv = fp8(psum - v8) -> compensated fp8 AV at
          0.5 cycles/col; moving strip broadcast on the slot dim.
  proj:   Y (bf16) x Wp (bf16), partial [C, T] f32 out; host reduces.
"""

import numpy as np
import ml_dtypes
from contextlib import ExitStack

B, T, C, H, D = 4, 2048, 1024, 16, 64
HL = H // 2          # 8 heads per core
N_CORES = 8
P = 128
NKS = C // 256       # 4 super contraction tiles of 256 (2 slots x 128)
NM = HL * D // P     # 4 m-tiles of Q/K head-dims
NS = T // P          # 16 s-tiles (key strips)
CH = 512             # t-chunk width
NCH = T // CH        # 4 t-chunks
E4 = ml_dtypes.float8_e4m3

AV_FP8 = True        # tier C: fp8 DoubleRow AV with V-residual slots

_nc_cache = None


def build_nc():
    global _nc_cache
    if _nc_cache is not None:
        return _nc_cache
    import concourse.bass as bass  # noqa: F401
    import concourse.tile as tile
    from concourse import bacc, mybir

    f32 = mybir.dt.float32
    f32r = mybir.dt.float32r
    bf16 = mybir.dt.bfloat16
    f8 = mybir.dt.float8e4
    DR = mybir.MatmulPerfMode.DoubleRow
    Exp = mybir.ActivationFunctionType.Exp
    vdt = f8 if AV_FP8 else bf16

    def mmdr(out, lhsT, rhs, **kw):
        nc.tensor.matmul(out, lhsT=lhsT, rhs=rhs, perf_mode=DR, **kw)

    nc = bacc.Bacc("TRN2", target_bir_lowering=False, debug=False,
                   enable_asserts=True, num_devices=N_CORES)
    x8f = nc.dram_tensor("x8f", (P, NKS * 2 * T), f8, kind="ExternalInput").ap()
    rxf = nc.dram_tensor("rxf", (P, NKS * 2 * T), f8, kind="ExternalInput").ap()
    wq8 = nc.dram_tensor("wq8", (P, 3 * NKS * 2 * HL * D), f8, kind="ExternalInput").ap()
    wk8 = nc.dram_tensor("wk8", (P, 3 * NKS * 2 * HL * D), f8, kind="ExternalInput").ap()
    wv8 = nc.dram_tensor("wv8", (P, 3 * NKS * 2 * HL * D), f8, kind="ExternalInput").ap()
    wpb = nc.dram_tensor("wpb", (P, NM * C), bf16, kind="ExternalInput").ap()
    trib = nc.dram_tensor("trib", (P, P), f32, kind="ExternalInput").ap()
    if AV_FP8:
        vones = nc.dram_tensor("vones", (P, NS * HL * 2 * 80), f8,
                               kind="ExternalInput").ap()
    else:
        vones = nc.dram_tensor("vones", (P, 2 * P), vdt, kind="ExternalInput").ap()
    ones1 = nc.dram_tensor("ones1", (P, D), f32r, kind="ExternalInput").ap()
    o = nc.dram_tensor("o", (C, T), f32, kind="ExternalOutput").ap()

    with tile.TileContext(nc) as tc:
        with ExitStack() as ctx:
            ctx.enter_context(nc.allow_low_precision(
                reason="fp8 DoubleRow with residual compensation; bf16 elsewhere"))

            const_pool = ctx.enter_context(tc.tile_pool(name="const", bufs=1))
            trib_sb = const_pool.tile([P, P], f32, name="trib_sb", tag="trib_sb")
            nc.sync.dma_start(out=trib_sb, in_=trib)
            ones1_sb = const_pool.tile([P, D], f32r, name="ones1_sb", tag="ones1_sb")
            nc.sync.dma_start(out=ones1_sb, in_=ones1)

            # persistent attention inputs
            att_pool = ctx.enter_context(tc.tile_pool(name="att", bufs=1))
            Q8 = [att_pool.tile([P, 2, T], f8, name=f"q8_{t}", tag=f"q8_{t}")
                  for t in range(2)]
            K8 = [att_pool.tile([P, 2, T], f8, name=f"k8_{t}", tag=f"k8_{t}")
                  for t in range(2)]
            if AV_FP8:
                # [s-in-tile, s-tile, head, slot(v8|rv), 80]: col D = rowsum
                # ones (32|0), cols D+1..79 = zero pad for the 16B-aligned
                # dual-fp8 ldweights stride; whole image loaded from host
                Vsb = att_pool.tile([P, NS, HL, 2, 80], f8, name="vsb", tag="vsb")
                nc.sync.dma_start(
                    out=Vsb,
                    in_=vones.rearrange("p (s h two d) -> p s h two d",
                                        s=NS, h=HL, two=2))
            else:
                Vsb = att_pool.tile([P, NS, HL, D + 1], bf16, name="vsb", tag="vsb")
                nc.sync.dma_start(out=Vsb[:, :, :, D],
                                  in_=vones[:, 0:NS * HL].rearrange(
                                      "p (s h) -> p s h", s=NS))
            yt = att_pool.tile([P, NM, T], bf16, name="yt", tag="yt")

            # ---- Phase 1: QKV projections (fp8 DoubleRow, 3-pass residual) ----
            with ExitStack() as p1:
                xpool = p1.enter_context(tc.tile_pool(name="xpool", bufs=1))
                wpool = p1.enter_context(tc.tile_pool(name="wpool", bufs=1))
                stpool = p1.enter_context(tc.tile_pool(name="stpool", bufs=3))
                qkv_ps = p1.enter_context(
                    tc.tile_pool(name="qkv_ps", bufs=3, space="PSUM"))

                x8sb = xpool.tile([P, NKS, 2, T], f8, name="x8sb", tag="x8sb")
                rxsb = xpool.tile([P, NKS, 2, T], f8, name="rxsb", tag="rxsb")
                x8v = x8f.rearrange("p (k two t) -> p k two t", k=NKS, two=2)
                rxv = rxf.rearrange("p (k two t) -> p k two t", k=NKS, two=2)
                for ch in range(NCH):
                    cs = slice(ch * CH, (ch + 1) * CH)
                    nc.sync.dma_start(out=x8sb[:, :, :, cs], in_=x8v[:, :, :, cs])
                    nc.sync.dma_start(out=rxsb[:, :, :, cs], in_=rxv[:, :, :, cs])
                W = []
                for name, src in (("wq", wq8), ("wk", wk8), ("wv", wv8)):
                    wsb = wpool.tile([P, 3, NKS, 2, HL * D], f8, name=name, tag=name)
                    nc.sync.dma_start(
                        out=wsb,
                        in_=src.rearrange("p (pss k two m) -> p pss k two m",
                                          pss=3, k=NKS, two=2))
                    W.append(wsb)

                def qkv_mms(ps, wsb, stat_x, mov_is_w, ms):
                    # 3 passes x NKS super tiles; pass 1 uses rx, passes 0/2 use x8
                    first = True
                    for k in range(NKS):
                        for pss in range(3):
                            xs = rxsb if pss == 1 else x8sb
                            last = (k == NKS - 1 and pss == 2)
                            if mov_is_w:
                                mmdr(ps, xs[:, k, :, stat_x], wsb[:, pss, k, :, :],
                                     start=first, stop=last)
                            else:
                                mmdr(ps, wsb[:, pss, k, :, ms], xs[:, k, :, stat_x],
                                     start=first, stop=last)
                            first = False

                for ch in range(NCH):
                    cs = slice(ch * CH, (ch + 1) * CH)
                    for proj in range(2):           # Q, K
                        dst = Q8 if proj == 0 else K8
                        for mt in range(NM):
                            ps = qkv_ps.tile([P, CH], f32, name="qk_ps", tag="qkv")
                            qkv_mms(ps, W[proj], cs, False,
                                    slice(mt * P, (mt + 1) * P))
                            stg = stpool.tile([P, CH], f8, name="stg", tag="stg")
                            nc.vector.tensor_copy(stg, ps)
                            # fold [ (h i p) t ] -> [ h two p t ] at base 64*(mt%2)
                            src = stg.rearrange("(h i p) t -> h i p t", h=2, i=2, p=32)
                            dtile = dst[mt // 2]
                            dv = dtile[64 * (mt % 2):64 * (mt % 2) + 64, :, cs]
                            dv = dv.rearrange("(h p) two t -> h two p t", h=2, p=32)
                            for hh in range(2):
                                for sl2 in range(2):
                                    nc.sync.dma_start(out=dv[hh, sl2],
                                                      in_=src[hh, sl2])
                    for sl in range(CH // P):       # V s-tiles
                        s = ch * (CH // P) + sl
                        ts = slice(ch * CH + sl * P, ch * CH + (sl + 1) * P)
                        ps = qkv_ps.tile([P, HL * D], f32, name="v_ps", tag="qkv")
                        qkv_mms(ps, W[2], ts, True, None)
                        pr = ps.rearrange("p (h d) -> p h d", h=HL)
                        if AV_FP8:
                            nc.vector.tensor_copy(Vsb[:, s, :, 0, 0:D], pr)
                            nc.vector.tensor_sub(Vsb[:, s, :, 1, 0:D], pr,
                                                 Vsb[:, s, :, 0, 0:D])
                        else:
                            nc.vector.tensor_copy(Vsb[:, s, :, 0:D], pr)

            # ---- Phase 2: attention, globally software-pipelined ----
            with ExitStack() as p2:
                strip_pool = p2.enter_context(tc.tile_pool(name="strips", bufs=8))
                small = p2.enter_context(tc.tile_pool(name="small", bufs=3))
                sc_ps = p2.enter_context(
                    tc.tile_pool(name="sc_ps", bufs=2, space="PSUM"))
                av_ps = p2.enter_context(
                    tc.tile_pool(name="av_ps", bufs=3, space="PSUM"))
                rps_ps = p2.enter_context(
                    tc.tile_pool(name="rps_ps", bufs=1, space="PSUM"))

                def make_pass(h, half):
                    qt, base = h // 4, 32 * (h % 4)
                    tlo = half * 1024
                    ns = 8 if half == 0 else NS
                    st = {"strips": [None] * ns, "avs": None}

                    def do_scores(i):
                        t0 = P * i
                        s0 = max(t0, tlo)
                        strip = strip_pool.tile([P, 1024], vdt,
                                                name="strip", tag="strip")
                        st["strips"][i] = strip
                        ps = sc_ps.tile([P, 1024], f32, name="sc_ps", tag="sc")
                        b0 = s0
                        while b0 < tlo + 1024:
                            b1 = min((b0 // CH + 1) * CH, tlo + 1024)
                            mmdr(ps[:, b0 - tlo:b1 - tlo],
                                 K8[qt][base:base + 32, :, t0:t0 + P],
                                 Q8[qt][base:base + 32, :, b0:b1],
                                 start=True, stop=True,
                                 tile_position=(base, 0))
                            b0 = b1
                        if t0 >= tlo:   # causal mask: additive -1e6 pre-exp
                            nc.vector.tensor_add(
                                ps[:, t0 - tlo:t0 - tlo + P],
                                ps[:, t0 - tlo:t0 - tlo + P], trib_sb)
                        nc.scalar.activation(
                            strip[:, s0 - tlo:1024], ps[:, s0 - tlo:1024],
                            Exp, scale=1.0 / 8192.0)

                    def do_av(i):
                        if st["avs"] is None:
                            avp = 80 if AV_FP8 else D + 1
                            st["avs"] = {j: av_ps.tile([avp, CH], f32,
                                                       name=f"av{j}", tag="av")
                                         for j in (2 * half, 2 * half + 1)}
                        avs = st["avs"]
                        t0 = P * i
                        strip = st["strips"][i]
                        for j in (2 * half, 2 * half + 1):
                            if CH * (j + 1) <= t0:
                                continue
                            ts0 = max(CH * j, t0)
                            w = CH * (j + 1) - ts0
                            if AV_FP8:
                                mv = strip[:, None, ts0 - tlo:CH * (j + 1) - tlo]
                                mv = mv.broadcast_to((P, 2, w))
                                mmdr(avs[j][:, ts0 - CH * j:CH],
                                     Vsb[:, i, h, :, :], mv,
                                     start=(i == 0), stop=(i == 4 * j + 3),
                                     skip_group_check=True)
                            else:
                                nc.tensor.matmul(
                                    avs[j][:, ts0 - CH * j:CH],
                                    lhsT=Vsb[:, i, h, :],
                                    rhs=strip[:, ts0 - tlo:CH * (j + 1) - tlo],
                                    start=(i == 0), stop=(i == 4 * j + 3),
                                    skip_group_check=True)
                        if i % 4 == 3 and i // 4 in avs:
                            j = i // 4
                            rec = small.tile([D + 1, CH], f32r, name="rec", tag="rec")
                            nc.vector.reciprocal(rec[D:D + 1, :], avs[j][D:D + 1, :])
                            rps = rps_ps.tile([D, CH], f32, name="rps", tag="rps")
                            nc.tensor.matmul(rps, lhsT=ones1_sb[D:D + 1, 0:D],
                                             rhs=rec[D:D + 1, :],
                                             start=True, stop=True)
                            rsb = small.tile([D, CH], f32r, name="rsb", tag="rsb")
                            nc.vector.tensor_copy(rsb, rps)
                            # normalized head outputs straight into yt (bf16),
                            # cross-partition write for odd heads
                            nc.vector.tensor_mul(
                                yt[D * (h % 2):D * (h % 2) + D, h // 2,
                                   CH * j:CH * (j + 1)],
                                avs[j][0:D, :], rsb)

                    return ([lambda i=i: do_scores(i) for i in range(ns)],
                            [lambda i=i: do_av(i) for i in range(ns)])

                sflat, aflat = [], []
                for h in range(HL):
                    for half in range(2):
                        sc, ac = make_pass(h, half)
                        sflat += sc
                        aflat += ac
                LAG = 6
                for idx in range(len(sflat) + LAG):
                    if idx < len(sflat):
                        sflat[idx]()
                    if idx >= LAG:
                        aflat[idx - LAG]()

            # ---- Phase 3: output projection (partial; host adds bias+reduce) ----
            with ExitStack() as p3:
                wppool = p3.enter_context(tc.tile_pool(name="wppool", bufs=1))
                obpool = p3.enter_context(tc.tile_pool(name="obpool", bufs=3))
                pj_ps = p3.enter_context(
                    tc.tile_pool(name="pj_ps", bufs=2, space="PSUM"))
                Wp_sb = wppool.tile([P, NM, C], bf16, name="wp", tag="wp")
                nc.sync.dma_start(
                    out=Wp_sb, in_=wpb.rearrange("p (j c) -> p j c", j=NM))
                for ct in range(C // P):
                    ob = obpool.tile([P, T], f32, name="ob", tag="ob")
                    for ch in range(NCH):
                        ps = pj_ps.tile([P, CH], f32, name="p_ps", tag="pj")
                        for j in range(NM):
                            nc.tensor.matmul(
                                ps, lhsT=Wp_sb[:, j, ct * P:(ct + 1) * P],
                                rhs=yt[:, j, ch * CH:(ch + 1) * CH],
                                start=(j == 0), stop=(j == NM - 1))
                        if ch % 2 == 0:
                            nc.vector.tensor_copy(ob[:, ch * CH:(ch + 1) * CH], ps)
                        else:
                            nc.scalar.copy(ob[:, ch * CH:(ch + 1) * CH], ps)
                    nc.sync.dma_start(out=o[ct * P:(ct + 1) * P, :], in_=ob)

    nc.compile()
    _nc_cache = nc
    return nc


def _fp8(a):
    return np.asarray(a, dtype=E4)


def _prep_w(Wh):
    """W [C, M] f32 -> packed [128, 3, NKS, 2, M] fp8 (W8, W8h, rWh)."""
    W32 = 32.0 * Wh
    W8 = _fp8(W32)
    W8f = W8.astype(np.float32)
    W8h = _fp8(W8f / 32.0)
    rWh = _fp8(W32 - W8f)
    out = np.empty((P, 3, NKS, 2, Wh.shape[1]), dtype=E4)
    for arr, pss in ((W8, 0), (W8h, 1), (rWh, 2)):
        v = arr.reshape(NKS, 2, P, Wh.shape[1])
        out[:, pss] = v.transpose(2, 0, 1, 3)
    return out.reshape(P, -1)


def make_in_maps(x, Wq, Wk, Wv, Wp):
    """Shard FULL inputs into per-core input maps (all fp8/bf16 prep here)."""
    trib = np.where(np.arange(P)[None, :] >= np.arange(P)[:, None],
                    0.0, -1e6).astype(np.float32)
    if AV_FP8:
        vones = np.zeros((P, NS, HL, 2, 80), dtype=E4)
        vones[:, :, :, 0, D] = 32.0
        vones = vones.reshape(P, -1)
    else:
        vones = np.zeros((P, 2 * P), dtype=ml_dtypes.bfloat16)
        vones[:, 0:P] = 32.0
    ones1 = np.ones((P, D), dtype=np.float32)
    in_maps = []
    for c in range(N_CORES):
        b, g = c // 2, c % 2
        hs = slice(g * HL, (g + 1) * HL)
        xT = np.ascontiguousarray(x[b].T)           # [C, T]
        x8 = _fp8(xT)
        rx = _fp8(32.0 * (xT - x8.astype(np.float32)))
        x8p = x8.reshape(NKS, 2, P, T).transpose(2, 0, 1, 3).reshape(P, -1)
        rxp = rx.reshape(NKS, 2, P, T).transpose(2, 0, 1, 3).reshape(P, -1)
        wq = Wq[hs].transpose(1, 0, 2).reshape(C, HL * D)
        wk = Wk[hs].transpose(1, 0, 2).reshape(C, HL * D)
        wv = Wv[hs].transpose(1, 0, 2).reshape(C, HL * D)
        wps = Wp[:, g * HL * D:(g + 1) * HL * D].T  # [512, C]
        wpb = np.ascontiguousarray(
            wps.reshape(NM, P, C).transpose(1, 0, 2).reshape(P, -1)
        ).astype(ml_dtypes.bfloat16)
        m = {
            "x8f": np.ascontiguousarray(x8p),
            "rxf": np.ascontiguousarray(rxp),
            "wq8": _prep_w(wq),
            "wk8": _prep_w(wk),
            "wv8": _prep_w(wv),
            "wpb": wpb,
            "trib": trib,
            "vones": vones,
            "ones1": ones1,
        }
        in_maps.append(m)
    return in_maps


def assemble(results, bp):
    """Sum head-group partials per batch, add bias, transpose back."""
    out = np.empty((B, T, C), dtype=np.float32)
    for b in range(B):
        acc = results[2 * b]["o"] + results[2 * b + 1]["o"]  # [C, T]
        out[b] = acc.T + bp[None, :]
    return out


def kernel(x, Wq, Wk, Wv, Wp, bp):
    from concourse import bass_utils
    x = np.asarray(x, dtype=np.float32)
    nc = build_nc()
    in_maps = make_in_maps(np.asarray(x), np.asarray(Wq), np.asarray(Wk),
                           np.asarray(Wv), np.asarray(Wp))
    res = bass_utils.run_bass_kernel_spmd(nc, in_maps, core_ids=list(range(N_CORES)))
    return assemble(res.results, np.asarray(bp))


# revision 39
# speedup vs baseline: 1.5238x; 1.5238x over previous
"""Multi-head causal attention (B=4, T=2048, C=1024, H=16, D=64) on 8 trn2 cores.

Sharding: core c owns batch b = c//2 and heads g*8..g*8+7 where g = c%2
(batch-parallel x head-tensor-parallel). Each core computes its 8 heads'
QKV projections, causal attention, and a partial output projection
(columns of Wp belonging to its heads). Host sums the two head-group
partials per batch and adds the bias.

Precision/throughput scheme (fp8 = e4m3, DoubleRow = 2 contraction rows
per partition at 0.5 PE cycles/out-col):
  QKV:    psum = x8.W8 + rx.W8h + x8.rWh   (3-pass residual-compensated
          fp8 DoubleRow over 256-deep super k-tiles; W8 = fp8(32W),
          rx = fp8(32(x-x8)), W8h = fp8(W8/32), rWh = fp8(32W - W8))
          -> psum ~= 32 * (x . W), near-bf16 accuracy at 6/8 of f32r cost.
  scores: q8 = fp8(qkv_psum) [~32q]; per-strip DoubleRow with d=64 folded
          to [32 partitions x 2 slots]; exp scale 1/8192 recovers
          softmax(q.k/sqrt(D)). Causal masking via additive -1e6 on the
          diagonal psum block BEFORE exp (no post-exp mask multiply).
  AV:     tier B: strips+V in bf16, [V|32] stationary (rowsum via the
          32-valued ones column; matches the 32x v scale).
          tier C (AV_FP8): strips fp8; stationary DoubleRow slots
          (v8, rv) with rv = fp8(psum - v8) -> compensated fp8 AV at
          0.5 cycles/col; moving strip broadcast on the slot dim.
  proj:   Y (bf16) x Wp (bf16), partial [C, T] f32 out; host reduces.
"""

import numpy as np
import ml_dtypes
from contextlib import ExitStack

B, T, C, H, D = 4, 2048, 1024, 16, 64
HL = H // 2          # 8 heads per core
N_CORES = 8
P = 128
NKS = C // 256       # 4 super contraction tiles of 256 (2 slots x 128)
NM = HL * D // P     # 4 m-tiles of Q/K head-dims
NS = T // P          # 16 s-tiles (key strips)
CH = 512             # t-chunk width
NCH = T // CH        # 4 t-chunks
E4 = ml_dtypes.float8_e4m3

AV_FP8 = True        # tier C: fp8 DoubleRow AV with V-residual slots

_nc_cache = None


def build_nc():
    global _nc_cache
    if _nc_cache is not None:
        return _nc_cache
    import concourse.bass as bass  # noqa: F401
    import concourse.tile as tile
    from concourse import bacc, mybir

    f32 = mybir.dt.float32
    f32r = mybir.dt.float32r
    bf16 = mybir.dt.bfloat16
    f8 = mybir.dt.float8e4
    DR = mybir.MatmulPerfMode.DoubleRow
    Exp = mybir.ActivationFunctionType.Exp
    vdt = f8 if AV_FP8 else bf16

    def mmdr(out, lhsT, rhs, **kw):
        nc.tensor.matmul(out, lhsT=lhsT, rhs=rhs, perf_mode=DR, **kw)

    nc = bacc.Bacc("TRN2", target_bir_lowering=False, debug=False,
                   enable_asserts=True, num_devices=N_CORES)
    x8f = nc.dram_tensor("x8f", (P, NKS * 2 * T), f8, kind="ExternalInput").ap()
    rxf = nc.dram_tensor("rxf", (P, NKS * 2 * T), f8, kind="ExternalInput").ap()
    wq8 = nc.dram_tensor("wq8", (P, 3 * NKS * 2 * HL * D), f8, kind="ExternalInput").ap()
    wk8 = nc.dram_tensor("wk8", (P, 3 * NKS * 2 * HL * D), f8, kind="ExternalInput").ap()
    wv8 = nc.dram_tensor("wv8", (P, 3 * NKS * 2 * HL * D), f8, kind="ExternalInput").ap()
    wpb = nc.dram_tensor("wpb", (P, NM * C), bf16, kind="ExternalInput").ap()
    trib = nc.dram_tensor("trib", (P, P), f32, kind="ExternalInput").ap()
    tri01 = nc.dram_tensor("tri01", (P, P), vdt, kind="ExternalInput").ap()
    if AV_FP8:
        vones = nc.dram_tensor("vones", (P, 2 * P), f8,
                               kind="ExternalInput").ap()
    else:
        vones = nc.dram_tensor("vones", (P, 2 * P), vdt, kind="ExternalInput").ap()
    ones1 = nc.dram_tensor("ones1", (P, D), f32r, kind="ExternalInput").ap()
    o = nc.dram_tensor("o", (C, T), f32, kind="ExternalOutput").ap()

    with tile.TileContext(nc) as tc:
        with ExitStack() as ctx:
            ctx.enter_context(nc.allow_low_precision(
                reason="fp8 DoubleRow with residual compensation; bf16 elsewhere"))

            const_pool = ctx.enter_context(tc.tile_pool(name="const", bufs=1))
            trib_sb = const_pool.tile([P, P], f32, name="trib_sb", tag="trib_sb")
            nc.sync.dma_start(out=trib_sb, in_=trib)
            tri01_sb = const_pool.tile([P, P], vdt, name="tri01_sb", tag="tri01_sb")
            nc.sync.dma_start(out=tri01_sb, in_=tri01)
            ones1_sb = const_pool.tile([P, D], f32r, name="ones1_sb", tag="ones1_sb")
            nc.sync.dma_start(out=ones1_sb, in_=ones1)

            # persistent attention inputs
            att_pool = ctx.enter_context(tc.tile_pool(name="att", bufs=1))
            Q8 = [[att_pool.tile([P, 2, T // 2], f8, name=f"q8_{t}_{hf}",
                                 tag=f"q8_{t}_{hf}") for hf in range(2)]
                  for t in range(NM)]
            K8 = [[att_pool.tile([P, 2, T // 2], f8, name=f"k8_{t}_{hf}",
                                 tag=f"k8_{t}_{hf}") for hf in range(2)]
                  for t in range(NM)]
            if AV_FP8:
                # [s-in-tile, s-tile, head, slot(v8|rv), 80]: col D = rowsum
                # ones (32|0), cols D+1..79 = zero pad for the 16B-aligned
                # dual-fp8 ldweights stride; whole image loaded from host
                Vsb = att_pool.tile([P, NS, HL, 2, 80], f8, name="vsb", tag="vsb")
                nc.gpsimd.dma_start(
                    out=Vsb,
                    in_=vones.rearrange("p (s h two d) -> p s h two d",
                                        s=NS, h=HL, two=2))
            else:
                Vsb = att_pool.tile([P, NS, HL, D + 1], bf16, name="vsb", tag="vsb")
                nc.sync.dma_start(out=Vsb[:, :, :, D],
                                  in_=vones[:, 0:NS * HL].rearrange(
                                      "p (s h) -> p s h", s=NS))
            yt = [att_pool.tile([P, NM, T // 2], bf16, name=f"yt{hf}",
                                tag=f"yt{hf}") for hf in range(2)]

            # ---- Phase 1: QKV projections (fp8 DoubleRow, 3-pass residual) ----
            with ExitStack() as p1:
                xpool = p1.enter_context(tc.tile_pool(name="xpool", bufs=1))
                wpool = p1.enter_context(tc.tile_pool(name="wpool", bufs=1))
                stpool = p1.enter_context(tc.tile_pool(name="stpool", bufs=3))
                qkv_ps = p1.enter_context(
                    tc.tile_pool(name="qkv_ps", bufs=3, space="PSUM"))

                x8sb = xpool.tile([P, NKS, 2, T], f8, name="x8sb", tag="x8sb")
                rxsb = xpool.tile([P, NKS, 2, T], f8, name="rxsb", tag="rxsb")
                x8v = x8f.rearrange("p (k two t) -> p k two t", k=NKS, two=2)
                rxv = rxf.rearrange("p (k two t) -> p k two t", k=NKS, two=2)
                for ch in range(NCH):
                    cs = slice(ch * CH, (ch + 1) * CH)
                    nc.sync.dma_start(out=x8sb[:, :, :, cs], in_=x8v[:, :, :, cs])
                    nc.sync.dma_start(out=rxsb[:, :, :, cs], in_=rxv[:, :, :, cs])
                W = []
            for name in ("wq", "wk", "wv"):
                W.append(wpool.tile([P, 3, NKS, 2, HL * D], f8, name=name,
                                    tag=name))
            wsrc = {0: wq8, 1: wk8, 2: wv8}
            Wp_sb = wppool.tile([P, NM, C], bf16, name="wp", tag="wp")

            def load_w(i):
                nc.gpsimd.dma_start(
                    out=W[i],
                    in_=wsrc[i].rearrange("p (pss k two m) -> p pss k two m",
                                          pss=3, k=NKS, two=2))

            def load_x(ch):
                cs = slice(ch * CH, (ch + 1) * CH)
                nc.scalar.dma_start(out=x8t[ch], in_=x8v[:, :, :, cs])
                nc.scalar.dma_start(out=rxt[ch], in_=rxv[:, :, :, cs])

            # dependency-ordered input loads: only what the upfront units
            # and first passes need; the rest is deferred into the slot loop
            load_w(0)
            load_x(0)
            load_w(1)
            load_w(2)
            load_x(1)

            # PE clock warm-up: a back-to-back dummy matmul chain on the tiny
            # const tile keeps the PE continuously busy through the input-DMA
            # window so the p-state ramp completes before the first real unit.
            wu = mix_ps.tile([D, D], f32, name="wu", tag="mix")
            for _ in range(48):
                nc.tensor.matmul(wu, lhsT=ones1_sb[:, 0:D], rhs=ones1_sb[:, 0:D],
                                 start=True, stop=True)
            # preload the Exp activation table off the critical path
            wua = small.tile([1, 2], f32, name="wua", tag="rec")
            nc.scalar.activation(wua, ones1_sb[0:1, 0:2], Exp, scale=1.0)

            def load_vsb():
                if AV_FP8:
                    # init via Pool-engine memsets: ones col (32|0) + zero
                    # pads; avoids DMAing a 2.6MB mostly-zero image
                    nc.gpsimd.memset(Vsb[:, :, :, 0, D], 32.0)
                    nc.gpsimd.memset(Vsb[:, :, :, 0, D + 1:80], 0.0)
                    nc.gpsimd.memset(Vsb[:, :, :, 1, D:80], 0.0)
                else:
                    nc.gpsimd.dma_start(out=Vsb[:, :, :, D],
                                        in_=vones[:, 0:NS * HL].rearrange(
                                            "p (s h) -> p s h", s=NS))

            def load_wp():
                nc.gpsimd.dma_start(
                    out=Wp_sb, in_=wpb.rearrange("p (j c) -> p j c", j=NM))

            load_x(2)
            load_x(3)
            load_vsb()
            deferred = {30: load_wp}

            def qkv_mms(ps, wsb, stat_x, mov_is_w, ms):
                    # 3 passes x NKS super tiles; pass 1 uses rx, passes 0/2 use x8
                    first = True
                    for k in range(NKS):
                        for pss in range(3):
                            xs = rxsb if pss == 1 else x8sb
                            last = (k == NKS - 1 and pss == 2)
                            if mov_is_w:
                                mmdr(ps, xs[:, k, :, stat_x], wsb[:, pss, k, :, :],
                                     start=first, stop=last)
                            else:
                                mmdr(ps, wsb[:, pss, k, :, ms], xs[:, k, :, stat_x],
                                     start=first, stop=last)
                            first = False

                def qk_unit(proj, mt, ch):
                    cs = slice(ch * CH, (ch + 1) * CH)
                    dst = Q8 if proj == 0 else K8
                    ps = qkv_ps.tile([P, CH], f32, name="qk_ps", tag="qkv")
                    qkv_mms(ps, W[proj], cs, False, slice(mt * P, (mt + 1) * P))
                    stg = stpool.tile([P, CH], f8, name="stg", tag="stg")
                    nc.vector.tensor_copy(stg, ps)
                    # fold [ (h i p) t ] -> [ h two p t ]: heads 2mt,2mt+1 at
                    # bases 0/32 of the per-mt tile
                    src = stg.rearrange("(h i p) t -> h i p t", h=2, i=2, p=32)
                    dv = dst[mt][0:64, :, cs]
                    dv = dv.rearrange("(h p) two t -> h two p t", h=2, p=32)
                    for hh in range(2):
                        for sl2 in range(2):
                            nc.sync.dma_start(out=dv[hh, sl2], in_=src[hh, sl2])

                def v_unit(sidx):
                    ts = slice(sidx * P, (sidx + 1) * P)
                    ps = qkv_ps.tile([P, HL * D], f32, name="v_ps", tag="qkv")
                    qkv_mms(ps, W[2], ts, True, None)
                    pr = ps.rearrange("p (h d) -> p h d", h=HL)
                    if AV_FP8:
                        nc.vector.tensor_copy(Vsb[:, sidx, :, 0, 0:D], pr)
                        nc.vector.tensor_sub(Vsb[:, sidx, :, 1, 0:D], pr,
                                             Vsb[:, sidx, :, 0, 0:D])
                    else:
                        nc.vector.tensor_copy(Vsb[:, sidx, :, 0:D], pr)

                # upfront: m-tile 0 of Q,K plus all of V -> heads 0,1 can
                # start while the rest of QKV weaves into the attention
                # pipeline below
                for ch in range(NCH):
                    qk_unit(0, 0, ch)
                for ch in range(NCH):
                    qk_unit(1, 0, ch)
                for sidx in range(NS):
                    v_unit(sidx)
                extras = [lambda proj=proj, mt=mt, ch=ch: qk_unit(proj, mt, ch)
                          for mt in range(1, NM)
                          for proj in range(2)
                          for ch in range(NCH)]

            # ---- Phase 2: attention, globally software-pipelined ----
            with ExitStack() as p2:
                strip_pool = p2.enter_context(tc.tile_pool(name="strips", bufs=8))
                small = p2.enter_context(tc.tile_pool(name="small", bufs=3))
                sc_ps = p2.enter_context(
                    tc.tile_pool(name="sc_ps", bufs=2, space="PSUM"))
                av_ps = p2.enter_context(
                    tc.tile_pool(name="av_ps", bufs=2, space="PSUM"))
                rps_ps = p2.enter_context(
                    tc.tile_pool(name="rps_ps", bufs=1, space="PSUM"))

                def make_pass(h, half):
                    qt, base = h // 2, 32 * (h % 2)
                    tlo = half * 1024
                    ns = 8 if half == 0 else NS
                    st = {"strips": [None] * ns, "avs": None}

                    def do_scores(i):
                        t0 = P * i
                        s0 = max(t0, tlo)
                        strip = strip_pool.tile([P, 1024], vdt,
                                                name="strip", tag="strip")
                        st["strips"][i] = strip
                        ps = sc_ps.tile([P, 1024], f32, name="sc_ps", tag="sc")
                        b0 = s0
                        while b0 < tlo + 1024:
                            b1 = min((b0 // CH + 1) * CH, tlo + 1024)
                            mmdr(ps[:, b0 - tlo:b1 - tlo],
                                 K8[qt][base:base + 32, :, t0:t0 + P],
                                 Q8[qt][base:base + 32, :, b0:b1],
                                 start=True, stop=True,
                                 tile_position=(base, 0))
                            b0 = b1
                        if t0 >= tlo:   # causal mask: additive -1e6 pre-exp
                            nc.vector.tensor_add(
                                ps[:, t0 - tlo:t0 - tlo + P],
                                ps[:, t0 - tlo:t0 - tlo + P], trib_sb)
                        nc.scalar.activation(
                            strip[:, s0 - tlo:1024], ps[:, s0 - tlo:1024],
                            Exp, scale=1.0 / 8192.0)

                    def do_av(i):
                        if st["avs"] is None:
                            avp = 80 if AV_FP8 else D + 1
                            st["avs"] = {j: av_ps.tile([avp, CH], f32,
                                                       name=f"av{j}", tag="av")
                                         for j in (2 * half, 2 * half + 1)}
                        avs = st["avs"]
                        t0 = P * i
                        strip = st["strips"][i]
                        for j in (2 * half, 2 * half + 1):
                            if CH * (j + 1) <= t0:
                                continue
                            ts0 = max(CH * j, t0)
                            w = CH * (j + 1) - ts0
                            if AV_FP8:
                                mv = strip[:, None, ts0 - tlo:CH * (j + 1) - tlo]
                                mv = mv.broadcast_to((P, 2, w))
                                mmdr(avs[j][:, ts0 - CH * j:CH],
                                     Vsb[:, i, h, :, :], mv,
                                     start=(i == 0), stop=(i == 4 * j + 3),
                                     skip_group_check=True)
                            else:
                                nc.tensor.matmul(
                                    avs[j][:, ts0 - CH * j:CH],
                                    lhsT=Vsb[:, i, h, :],
                                    rhs=strip[:, ts0 - tlo:CH * (j + 1) - tlo],
                                    start=(i == 0), stop=(i == 4 * j + 3),
                                    skip_group_check=True)
                        if i % 4 == 3 and i // 4 in avs:
                            j = i // 4
                            rec = small.tile([D + 1, CH], f32r, name="rec", tag="rec")
                            nc.vector.reciprocal(rec[D:D + 1, :], avs[j][D:D + 1, :])
                            rps = rps_ps.tile([D, CH], f32, name="rps", tag="rps")
                            nc.tensor.matmul(rps, lhsT=ones1_sb[D:D + 1, 0:D],
                                             rhs=rec[D:D + 1, :],
                                             start=True, stop=True)
                            rsb = small.tile([D, CH], f32r, name="rsb", tag="rsb")
                            nc.vector.tensor_copy(rsb, rps)
                            # normalized head outputs straight into yt (bf16),
                            # cross-partition write for odd heads
                            nc.vector.tensor_mul(
                                yt[half][D * (h % 2):D * (h % 2) + D, h // 2,
                                         CH * (j % 2):CH * (j % 2) + CH],
                                avs[j][0:D, :], rsb)

                    return ([lambda i=i: do_scores(i) for i in range(ns)],
                            [lambda i=i: do_av(i) for i in range(ns)])

                sflat, aflat = [], []
                for h in range(HL):
                    for half in range(2):
                        sc, ac = make_pass(h, half)
                        sflat += sc
                        aflat += ac
                LAG = 8
                for idx in range(len(sflat) + LAG):
                    if idx < len(sflat):
                        sflat[idx]()
                    if idx >= LAG:
                        aflat[idx - LAG]()

            # ---- Phase 3: output projection (partial; host adds bias+reduce) ----
            with ExitStack() as p3:
                wppool = p3.enter_context(tc.tile_pool(name="wppool", bufs=1))
                obpool = p3.enter_context(tc.tile_pool(name="obpool", bufs=3))
                pj_ps = p3.enter_context(
                    tc.tile_pool(name="pj_ps", bufs=2, space="PSUM"))
                Wp_sb = wppool.tile([P, NM, C], bf16, name="wp", tag="wp")
                nc.sync.dma_start(
                    out=Wp_sb, in_=wpb.rearrange("p (j c) -> p j c", j=NM))
                for ct in range(C // P):
                    ob = obpool.tile([P, T], f32, name="ob", tag="ob")
                    for ch in range(NCH):
                        ps = pj_ps.tile([P, CH], f32, name="p_ps", tag="pj")
                        for j in range(NM):
                            nc.tensor.matmul(
                                ps, lhsT=Wp_sb[:, j, ct * P:(ct + 1) * P],
                                rhs=yt[:, j, ch * CH:(ch + 1) * CH],
                                start=(j == 0), stop=(j == NM - 1))
                        if ch % 2 == 0:
                            nc.vector.tensor_copy(ob[:, ch * CH:(ch + 1) * CH], ps)
                        else:
                            nc.scalar.copy(ob[:, ch * CH:(ch + 1) * CH], ps)
                    nc.sync.dma_start(out=o[ct * P:(ct + 1) * P, :], in_=ob)

    nc.compile()
    _nc_cache = nc
    return nc


def _fp8(a):
    return np.asarray(a, dtype=E4)


def _prep_w(Wh):
    """W [C, M] f32 -> packed [128, 3, NKS, 2, M] fp8 (W8, W8h, rWh)."""
    W32 = 32.0 * Wh
    W8 = _fp8(W32)
    W8f = W8.astype(np.float32)
    W8h = _fp8(W8f / 32.0)
    rWh = _fp8(W32 - W8f)
    out = np.empty((P, 3, NKS, 2, Wh.shape[1]), dtype=E4)
    for arr, pss in ((W8, 0), (W8h, 1), (rWh, 2)):
        v = arr.reshape(NKS, 2, P, Wh.shape[1])
        out[:, pss] = v.transpose(2, 0, 1, 3)
    return out.reshape(P, -1)


def make_in_maps(x, Wq, Wk, Wv, Wp):
    """Shard FULL inputs into per-core input maps (all fp8/bf16 prep here)."""
    trib = np.where(np.arange(P)[None, :] >= np.arange(P)[:, None],
                    0.0, -1e6).astype(np.float32)
    tdt = E4 if AV_FP8 else ml_dtypes.bfloat16
    tri01 = np.where(np.arange(P)[None, :] >= np.arange(P)[:, None],
                     1.0, 0.0).astype(tdt)
    if AV_FP8:
        vones = np.zeros((P, 2 * P), dtype=E4)
        vones[:, 0:P] = 32.0
    else:
        vones = np.zeros((P, 2 * P), dtype=ml_dtypes.bfloat16)
        vones[:, 0:P] = 32.0
    ones1 = np.ones((P, D), dtype=np.float32)
    in_maps = []
    for c in range(N_CORES):
        b, g = c // 2, c % 2
        hs = slice(g * HL, (g + 1) * HL)
        xT = np.ascontiguousarray(x[b].T)           # [C, T]
        x8 = _fp8(xT)
        rx = _fp8(32.0 * (xT - x8.astype(np.float32)))
        x8p = x8.reshape(NKS, 2, P, T).transpose(2, 0, 1, 3).reshape(P, -1)
        rxp = rx.reshape(NKS, 2, P, T).transpose(2, 0, 1, 3).reshape(P, -1)
        wq = Wq[hs].transpose(1, 0, 2).reshape(C, HL * D)
        wk = Wk[hs].transpose(1, 0, 2).reshape(C, HL * D)
        wv = Wv[hs].transpose(1, 0, 2).reshape(C, HL * D)
        wps = Wp[:, g * HL * D:(g + 1) * HL * D].T  # [512, C]
        wpb = np.ascontiguousarray(
            wps.reshape(NM, P, C).transpose(1, 0, 2).reshape(P, -1)
        ).astype(ml_dtypes.bfloat16)
        m = {
            "x8f": np.ascontiguousarray(x8p),
            "rxf": np.ascontiguousarray(rxp),
            "wq8": _prep_w(wq),
            "wk8": _prep_w(wk),
            "wv8": _prep_w(wv),
            "wpb": wpb,
            "trib": trib,
            "tri01": tri01,
            "vones": vones,
            "ones1": ones1,
        }
        in_maps.append(m)
    return in_maps


def assemble(results, bp):
    """Sum head-group partials per batch, add bias, transpose back."""
    out = np.empty((B, T, C), dtype=np.float32)
    for b in range(B):
        acc = results[2 * b]["o"] + results[2 * b + 1]["o"]  # [C, T]
        out[b] = acc.T + bp[None, :]
    return out


def kernel(x, Wq, Wk, Wv, Wp, bp):
    from concourse import bass_utils
    x = np.asarray(x, dtype=np.float32)
    nc = build_nc()
    in_maps = make_in_maps(np.asarray(x), np.asarray(Wq), np.asarray(Wk),
                           np.asarray(Wv), np.asarray(Wp))
    res = bass_utils.run_bass_kernel_spmd(nc, in_maps, core_ids=list(range(N_CORES)))
    return assemble(res.results, np.asarray(bp))
